# revision 6
# baseline (speedup 1.0000x reference)
"""Trainium2 Bass kernel for nn_Convolution_1451698946404 (GNN message passing).

Math:
  d[a,b]   = sqrt(||g_b - g_a||^2 + eps)
  rbf      = exp(-gamma_r (d - mu_r)^2) / sqrt(n_norm)
  out[a,i] = sum_{b,r} rbf[a,b,r] * (W_r @ feat_b)[i]

Fast path (uniform gamma, uniformly spaced mu — true for this problem's
setup_inputs): geometric-chain RBF evaluation.
  With s = gamma*d^2, P = exp(2*gamma*delta*d) and T_r = exp(-s) * P^r:
      rbf_r = T_r * exp(-gamma*mu_r^2) / sqrt(n_norm)
  so the per-r constants fold into the weights W_r on the host and the
  device computes only: one bf16 matmul for s (hi/lo-split augmented
  geometry, exact products), three ACT table functions from ONE table set
  (ln + exp: d = exp(0.5*ln(s)), avoiding a sqrt-table switch), and a
  short bf16 multiply chain for the powers of P. Two chain bases (r=0 and
  r=4, each an exact ACT exp) cap chain length at 3 to bound bf16 error.

Sharding (8 cores): 2 b-halves x 4 a-quarters. Each core contracts its
384 b's against its 192 a's for all 8 r; host adds the two b-half partial
outputs per a-quarter and concatenates quarters. No collectives.

Fallback path (arbitrary mu/gamma): the original per-r kernel (2 a-halves
x 4 r-groups, f32r distance matmul, tensor_scalar + square + exp per r).
"""

import os

import ml_dtypes
import numpy as np

import concourse.bass as bass
import concourse.tile as tile
from concourse import bacc, mybir
from concourse.bass import ts
from concourse.bass_utils import run_bass_kernel_spmd

N = 768
CIN = 16
COUT = 16
R = 8
NCORES = 8

F32 = mybir.dt.float32
F32R = mybir.dt.float32r
BF16 = mybir.dt.bfloat16
NPBF16 = ml_dtypes.bfloat16

_CACHE = {}
LAST_EXEC_NS = None
LAST_RESULTS = None


def _patch_act_tables():
    """Steer the act-table-load pass to the single set that holds BOTH ln
    and exp (natural_log_exp_and_others), so the kernel needs one table
    load instead of thrashing between the ln-only and exp-only sets. Set
    ids/ordering are preserved — only which sets the chooser may pick for
    ln/exp changes; the chosen set genuinely contains both functions."""
    import concourse.bacc as _bacc_mod

    if getattr(_bacc_mod, "_act_tables_patched", False):
        return
    orig = _bacc_mod.get_activation_tables
    Ln = mybir.ActivationFunctionType.Ln
    Exp = mybir.ActivationFunctionType.Exp

    def patched(arch):
        tables = dict(orig(arch))
        if any(
            Ln in fns and Exp in fns for name, fns in tables.items()
        ):
            tables = {
                name: (
                    fns
                    if (Ln in fns and Exp in fns)
                    else fns - {Ln, Exp}
                )
                for name, fns in tables.items()
            }
        return tables

    _bacc_mod.get_activation_tables = patched
    _bacc_mod._act_tables_patched = True


_patch_act_tables()

# ---------------------------------------------------------------------------
# Fast path: geometric chain (uniform gamma, uniform mu spacing)
# ---------------------------------------------------------------------------
BH = N // 2             # 384 b's per core (b-half)
AQ = N // 4             # 192 a's per core (a-quarter)
NBT = BH // 128         # 3 b-tiles
KAUG = 15               # 5 aug rows x (hi,hi,lo)/(hi,lo,hi) bf16 split
EPS_LN = 3e-4           # ln bias; > PE accumulation residue
RBASE = 4               # second chain base index


def _build_fast(mu0_zero: bool):
    nc = bacc.Bacc("TRN2", target_bir_lowering=False, debug=False)
    # geo = [augb(384) | auga(192)] hstacked, bf16
    geo = nc.dram_tensor("geo", [KAUG, BH + AQ], BF16, kind="ExternalInput")
    # fpw[b, t, r*16+i] = (feat @ (W_r * c_r / sqrt(n)).T)[b_global, i], bf16
    fpw = nc.dram_tensor("fpw", [128, NBT, R * COUT], BF16, kind="ExternalInput")
    # per-partition scalars: [kP, kBase, k0, pad]
    scol = nc.dram_tensor("scol", [128, 4], F32, kind="ExternalInput")
    outt = nc.dram_tensor("outt", [COUT, AQ], F32, kind="ExternalOutput")

    Exp = mybir.ActivationFunctionType.Exp
    Ln = mybir.ActivationFunctionType.Ln

    with tile.TileContext(nc) as tc:
        with (
            tc.tile_pool(name="const", bufs=1) as const,
            tc.tile_pool(name="psd", bufs=1, space="PSUM") as psd,
            tc.tile_pool(name="pso", bufs=1, space="PSUM") as pso,
        ):
            geo_sb = const.tile([KAUG, BH + AQ], BF16)
            fp_sb = const.tile([128, NBT, R * COUT], BF16)
            scol_sb = const.tile([128, 4], F32)
            eps_sb = const.tile([128, 1], F32)
            nc.vector.memset(eps_sb[:], EPS_LN)
            tall = const.tile([128, R, NBT, AQ], BF16)
            ln_sb = const.tile([128, NBT, AQ], F32)
            d_sb = const.tile([128, NBT, AQ], F32)
            u4_sb = const.tile([128, NBT, AQ], F32)
            p_sb = const.tile([128, NBT, AQ], BF16)
            res_sb = const.tile([COUT, AQ], F32)

            nc.sync.dma_start(out=geo_sb[:], in_=geo.ap())
            nc.gpsimd.dma_start(out=fp_sb[:], in_=fpw.ap())
            nc.gpsimd.dma_start(out=scol_sb[:], in_=scol.ap())

            # s = gamma * d^2 (b-side aug rows pre-scaled by gamma on host)
            # one padded PSUM bank per b-tile so each matmul stays in-bank
            s_ps = psd.tile([128, NBT, 512], F32)
            for t in range(NBT):
                nc.tensor.matmul(
                    out=s_ps[:, t, :AQ],
                    lhsT=geo_sb[:, ts(t, 128)],
                    rhs=geo_sb[:, BH:],
                    start=True,
                    stop=True,
                )
            s_view = s_ps[:, :, :AQ]

            # ACT passes, all from the natural_log_exp table set:
            # L = ln(s + eps); d = exp(0.5 L) = sqrt(s); P = exp(kP * d)
            nc.scalar.activation(out=ln_sb[:], in_=s_view, func=Ln, bias=eps_sb[:])
            if mu0_zero:
                # T_0 = exp(-s)
                nc.scalar.activation(out=tall[:, 0], in_=s_view, func=Exp, scale=-1.0)
            nc.scalar.activation(out=d_sb[:], in_=ln_sb[:], func=Exp, scale=0.5)
            nc.scalar.activation(
                out=p_sb[:], in_=d_sb[:], func=Exp, scale=scol_sb[:, 0:1]
            )
            # second base: T_4 = exp(kBase * d - s)
            nc.vector.scalar_tensor_tensor(
                out=u4_sb[:],
                in0=d_sb[:],
                scalar=scol_sb[:, 1:2],
                in1=s_view,
                op0=mybir.AluOpType.mult,
                op1=mybir.AluOpType.subtract,
            )
            nc.scalar.activation(out=tall[:, RBASE], in_=u4_sb[:], func=Exp)
            if not mu0_zero:
                # T_0 = exp(k0 * d - s)
                u0_sb = const.tile([128, NBT, AQ], F32)
                nc.vector.scalar_tensor_tensor(
                    out=u0_sb[:],
                    in0=d_sb[:],
                    scalar=scol_sb[:, 2:3],
                    in1=s_view,
                    op0=mybir.AluOpType.mult,
                    op1=mybir.AluOpType.subtract,
                )
                nc.scalar.activation(out=tall[:, 0], in_=u0_sb[:], func=Exp)

            # geometric chains from the two bases
            for dst, src in ((1, 0), (2, 1), (3, 2), (5, 4), (6, 5), (7, 6)):
                nc.vector.tensor_mul(tall[:, dst], tall[:, src], p_sb[:])

            # out^T[i, a] = sum_{r, b} fp[b, (r,i)] * T_r[b, a]
            out_ps = pso.tile([COUT, AQ], F32)
            k = 0
            for r in range(R):
                for t in range(NBT):
                    nc.tensor.matmul(
                        out=out_ps[:],
                        lhsT=fp_sb[:, t, ts(r, COUT)],
                        rhs=tall[:, r, t],
                        start=(k == 0),
                        stop=(k == R * NBT - 1),
                    )
                    k += 1

            nc.vector.tensor_copy(out=res_sb[:], in_=out_ps[:])
            nc.sync.dma_start(out=outt.ap(), in_=res_sb[:])

    nc.compile()
    return nc


def _split8(x):
    """Split x = hi + lo into two bf16 parts (products of parts are exact
    in f32)."""
    x = x.astype(np.float32)
    hi = x.astype(NPBF16).astype(np.float32)
    lo = (x - hi).astype(NPBF16).astype(np.float32)
    return hi, lo


def _kernel_fast(f, g, Wf, muf, gaf, nn):
    global LAST_EXEC_NS, LAST_RESULTS
    ga = float(gaf[0])
    delta = float(muf[1] - muf[0])
    mu0_zero = abs(float(muf[0])) < 1e-7

    key = ("fast", mu0_zero)
    if key not in _CACHE:
        _CACHE[key] = _build_fast(mu0_zero)
    nc = _CACHE[key]

    gs = g.astype(np.float32)
    sq = (gs * gs).sum(1)
    one = np.ones(N, np.float32)
    augb5 = np.stack([gs[:, 0], gs[:, 1], gs[:, 2], sq, one]) * np.float32(ga)
    auga5 = np.stack([-2 * gs[:, 0], -2 * gs[:, 1], -2 * gs[:, 2], one, sq])
    bh, bl = _split8(augb5)
    ah, al = _split8(auga5)
    augb = np.concatenate([bh, bh, bl], axis=0)          # [15, N]
    auga = np.concatenate([ah, al, ah], axis=0)

    # weights with per-r chain constants folded in
    # rbf_r = T_r * exp(-ga*(mu0 + r*delta)^2 + 2*ga*mu0*... ) — with the
    # chain T_r = T_0 * P^r and T_0 = exp(2*ga*mu0*d - s), the fold is
    # c_r = exp(-ga*mu_r^2 + ga*mu0^2 + ... ); derive directly:
    # T_r = exp(-ga*d^2 + 2*ga*(mu0 + r*delta)*d) = rbf_r * exp(ga*mu_r^2)
    # => c_r = exp(-ga*mu_r^2) / sqrt(n_norm)
    Wc = np.empty((R, COUT, CIN), np.float64)
    for r in range(R):
        c = np.exp(-ga * float(muf[r]) ** 2) / np.sqrt(nn)
        Wc[r] = Wf.astype(np.float64)[r] * c

    # fp[b_global, r, i] = feat @ Wc[r].T
    fp_full = np.einsum("bj,rij->bri", f.astype(np.float64), Wc).astype(np.float32)

    kP = np.float32(2.0 * delta * np.sqrt(ga))
    kBase = np.float32(2.0 * np.sqrt(ga) * float(muf[RBASE]))
    k0 = np.float32(2.0 * np.sqrt(ga) * float(muf[0]))
    scol = np.zeros((128, 4), np.float32)
    scol[:, 0] = kP
    scol[:, 1] = kBase
    scol[:, 2] = k0

    in_maps = []
    for h in range(2):
        fp_h = fp_full[h * BH : (h + 1) * BH].reshape(NBT, 128, R * COUT)
        fp_blob = np.ascontiguousarray(fp_h.transpose(1, 0, 2)).astype(NPBF16)
        augb_h = augb[:, h * BH : (h + 1) * BH]
        for q in range(4):
            geo = np.ascontiguousarray(
                np.concatenate([augb_h, auga[:, q * AQ : (q + 1) * AQ]], axis=1)
            ).astype(NPBF16)
            in_maps.append({"geo": geo, "fpw": fp_blob, "scol": scol})

    trace = os.environ.get("KERNEL_TRACE", "0") == "1"
    res = run_bass_kernel_spmd(nc, in_maps, core_ids=list(range(NCORES)), trace=trace)
    LAST_EXEC_NS = res.exec_time_ns
    LAST_RESULTS = res

    out = np.zeros((1, N, COUT), np.float32)
    for q in range(4):
        acc = res.results[q]["outt"].astype(np.float64) + res.results[4 + q][
            "outt"
        ].astype(np.float64)
        out[0, q * AQ : (q + 1) * AQ, :] = acc.T.astype(np.float32)
    return out


# ---------------------------------------------------------------------------
# Fallback path: original per-r kernel (arbitrary mu/gamma)
# ---------------------------------------------------------------------------
AHALF = N // 2          # 384 output points per a-half
RPC = 2                 # radial bases per core
NBT6 = N // 128         # 6 b-tiles
KAUG15 = 15
EPS_BIAS = 3e-5


def _build_legacy():
    nc = bacc.Bacc("TRN2", target_bir_lowering=False, debug=False)
    geo = nc.dram_tensor("geo", [KAUG15, N + AHALF], F32, kind="ExternalInput")
    feats = nc.dram_tensor("feats", [CIN, N + RPC * COUT], F32, kind="ExternalInput")
    scols = nc.dram_tensor("scols", [128, 2 * RPC], F32, kind="ExternalInput")
    outt = nc.dram_tensor("outt", [COUT, AHALF], F32, kind="ExternalOutput")

    with tile.TileContext(nc) as tc:
        with (
            tc.tile_pool(name="const", bufs=1) as const,
            tc.tile_pool(name="work", bufs=3) as work,
            tc.tile_pool(name="psd", bufs=2, space="PSUM") as psd,
            tc.tile_pool(name="psf", bufs=1, space="PSUM") as psf,
            tc.tile_pool(name="pso", bufs=1, space="PSUM") as pso,
        ):
            geo_sb = const.tile([KAUG15, N + AHALF], F32R)
            feats_sb = const.tile([CIN, N + RPC * COUT], F32)
            scols_sb = const.tile([128, 2 * RPC], F32)
            eps_sb = const.tile([128, 1], F32)
            nc.vector.memset(eps_sb[:], EPS_BIAS)
            nc.sync.dma_start(out=geo_sb[:], in_=geo.ap().bitcast(F32R))
            nc.scalar.dma_start(out=feats_sb[:], in_=feats.ap())
            nc.scalar.dma_start(out=scols_sb[:], in_=scols.ap())
            augb_sb = geo_sb[:, :N]
            auga_sb = geo_sb[:, N:]
            featt_sb = feats_sb[:, :N]
            wtg_sb = feats_sb[:, N:]

            d_sb = const.tile([128, NBT6, AHALF], F32)
            for tp in range(NBT6 // 2):
                d2_ps = psd.tile([128, 2, 512], F32)
                for j in range(2):
                    nc.tensor.matmul(
                        out=d2_ps[:, j, :AHALF],
                        lhsT=augb_sb[:, ts(2 * tp + j, 128)],
                        rhs=auga_sb[:],
                        start=True,
                        stop=True,
                    )
                nc.scalar.activation(
                    out=d_sb[:, 2 * tp : 2 * tp + 2, :],
                    in_=d2_ps[:, :, :AHALF],
                    func=mybir.ActivationFunctionType.Sqrt,
                    bias=eps_sb[:],
                    scale=1.0,
                )

            fp_ps = psf.tile([128, NBT6, RPC * COUT], F32)
            for t in range(NBT6):
                nc.tensor.matmul(
                    out=fp_ps[:, t, :],
                    lhsT=featt_sb[:, ts(t, 128)],
                    rhs=wtg_sb[:],
                    start=True,
                    stop=True,
                )
            fp_sb = const.tile([128, NBT6, RPC * COUT], BF16)
            nc.vector.tensor_copy(out=fp_sb[:], in_=fp_ps[:])

            out_ps = pso.tile([COUT, AHALF], F32)
            k = 0
            for tp in range(NBT6 // 2):
                t_bf = work.tile([128, 2, RPC, AHALF], BF16, tag="t_bf")
                for rl in range(RPC):
                    nc.vector.tensor_scalar(
                        out=t_bf[:, :, rl, :],
                        in0=d_sb[:, 2 * tp : 2 * tp + 2, :],
                        scalar1=scols_sb[:, 2 * rl : 2 * rl + 1],
                        scalar2=scols_sb[:, 2 * rl + 1 : 2 * rl + 2],
                        op0=mybir.AluOpType.subtract,
                        op1=mybir.AluOpType.mult,
                    )
                q_bf = work.tile([128, 2, RPC, AHALF], BF16, tag="q_bf")
                nc.vector.tensor_mul(q_bf[:], t_bf[:], t_bf[:])
                rbf = work.tile([128, 2, RPC, AHALF], BF16, tag="rbf")
                nc.scalar.activation(
                    out=rbf[:],
                    in_=q_bf[:],
                    func=mybir.ActivationFunctionType.Exp,
                    scale=-1.0,
                )
                for j in range(2):
                    for rl in range(RPC):
                        nc.tensor.matmul(
                            out=out_ps[:],
                            lhsT=fp_sb[:, 2 * tp + j, ts(rl, COUT)],
                            rhs=rbf[:, j, rl, :],
                            start=(k == 0),
                            stop=(k == NBT6 * RPC - 1),
                        )
                        k += 1

            res_sb = const.tile([COUT, AHALF], F32)
            nc.vector.tensor_copy(out=res_sb[:], in_=out_ps[:])
            nc.sync.dma_start(out=outt.ap(), in_=res_sb[:])

    nc.compile()
    return nc


def _split12(x):
    """Veltkamp split: x = hi + lo with hi having <=12 significant bits."""
    x = x.astype(np.float32)
    c = (np.float32(2.0**12 + 1.0) * x).astype(np.float32)
    hi = (c - (c - x).astype(np.float32)).astype(np.float32)
    lo = (x - hi).astype(np.float32)
    return hi, lo


def _kernel_legacy(f, g, Wf, muf, gaf, nn):
    global LAST_EXEC_NS, LAST_RESULTS
    sq = (g * g).sum(1, dtype=np.float32)
    one = np.ones(N, np.float32)
    augb5 = np.stack([g[:, 0], g[:, 1], g[:, 2], sq, one])
    auga5_full = np.stack([-2 * g[:, 0], -2 * g[:, 1], -2 * g[:, 2], one, sq])
    bh, bl = _split12(augb5)
    ah, al = _split12(auga5_full)
    augb = np.concatenate([bh, bh, bl], axis=0)
    auga_full = np.concatenate([ah, al, ah], axis=0)
    featt = np.ascontiguousarray(f.T)
    Wn = (Wf.astype(np.float64) / np.sqrt(nn)).astype(np.float32)
    sg = np.sqrt(gaf)

    if "legacy" not in _CACHE:
        _CACHE["legacy"] = _build_legacy()
    nc = _CACHE["legacy"]

    in_maps = []
    for c in range(NCORES):
        h, grp = c // 4, c % 4
        rr = [RPC * grp + j for j in range(RPC)]
        wtg = np.ascontiguousarray(
            np.concatenate([Wn[r].T for r in rr], axis=1).astype(np.float32)
        )
        scol_vals = []
        for r in rr:
            scol_vals += [muf[r], sg[r]]
        scols = np.ascontiguousarray(
            np.tile(np.asarray(scol_vals, np.float32), (128, 1))
        )
        geo = np.ascontiguousarray(
            np.concatenate([augb, auga_full[:, h * AHALF : (h + 1) * AHALF]], axis=1)
        )
        feats_blob = np.ascontiguousarray(np.concatenate([featt, wtg], axis=1))
        in_maps.append({"geo": geo, "feats": feats_blob, "scols": scols})

    trace = os.environ.get("KERNEL_TRACE", "0") == "1"
    res = run_bass_kernel_spmd(nc, in_maps, core_ids=list(range(NCORES)), trace=trace)
    LAST_EXEC_NS = res.exec_time_ns
    LAST_RESULTS = res

    out = np.zeros((1, N, COUT), np.float32)
    for h in range(2):
        acc = np.zeros((COUT, AHALF), np.float64)
        for grp in range(4):
            acc += res.results[h * 4 + grp]["outt"].astype(np.float64)
        out[0, h * AHALF : (h + 1) * AHALF, :] = acc.T.astype(np.float32)
    return out


def kernel(features, geometry, W, mu, gamma, n_norm):
    f = np.ascontiguousarray(np.asarray(features, np.float32)[0])      # [N, CIN]
    g = np.ascontiguousarray(np.asarray(geometry, np.float32)[0])      # [N, 3]
    Wf = np.asarray(W, np.float32)                                     # [R, COUT, CIN]
    muf = np.asarray(mu, np.float64)
    gaf = np.asarray(gamma, np.float64)
    nn = float(np.asarray(n_norm))

    diffs = np.diff(muf)
    uniform = (
        gaf.min() > 0
        and np.ptp(gaf) <= 1e-5 * abs(gaf[0])
        and diffs.min() > 0
        and np.ptp(diffs) <= 1e-4 * abs(diffs[0])
    )
    if uniform:
        return _kernel_fast(f, g, Wf, muf, gaf, nn)
    return _kernel_legacy(f, g, Wf, muf, gaf, nn)


# revision 10
# speedup vs baseline: 1.1677x; 1.1677x over previous
"""Trainium2 Bass kernel for nn_Convolution_1451698946404 (GNN message passing).

Math:
  d[a,b]   = sqrt(||g_b - g_a||^2 + eps)
  rbf      = exp(-gamma_r (d - mu_r)^2) / sqrt(n_norm)
  out[a,i] = sum_{b,r} rbf[a,b,r] * (W_r @ feat_b)[i]

Fast path (uniform gamma, uniformly spaced mu — true for this problem's
setup_inputs): geometric-chain RBF evaluation.
  With s = gamma*d^2, P = exp(2*gamma*delta*d) and T_r = exp(-s) * P^r:
      rbf_r = T_r * exp(-gamma*mu_r^2) / sqrt(n_norm)
  so the per-r constants fold into the weights W_r on the host and the
  device computes only: one bf16 matmul for s (hi/lo-split augmented
  geometry, exact products), three ACT table functions from ONE table set
  (ln + exp: d = exp(0.5*ln(s)), avoiding a sqrt-table switch), and a
  short bf16 multiply chain for the powers of P. Two chain bases (r=0 and
  r=4, each an exact ACT exp) cap chain length at 3 to bound bf16 error.

Sharding (8 cores): 2 b-halves x 4 a-quarters. Each core contracts its
384 b's against its 192 a's for all 8 r; host adds the two b-half partial
outputs per a-quarter and concatenates quarters. No collectives.

Fallback path (arbitrary mu/gamma): the original per-r kernel (2 a-halves
x 4 r-groups, f32r distance matmul, tensor_scalar + square + exp per r).
"""

import os

import ml_dtypes
import numpy as np

import concourse.bass as bass
import concourse.tile as tile
from concourse import bacc, mybir
from concourse.bass import ts
from concourse.bass_utils import run_bass_kernel_spmd

N = 768
CIN = 16
COUT = 16
R = 8
NCORES = 8

F32 = mybir.dt.float32
F32R = mybir.dt.float32r
BF16 = mybir.dt.bfloat16
NPBF16 = ml_dtypes.bfloat16

_CACHE = {}
LAST_EXEC_NS = None
LAST_RESULTS = None


def _patch_act_tables():
    """Steer the act-table-load pass to the single set that holds BOTH ln
    and exp (natural_log_exp_and_others), so the kernel needs one table
    load instead of thrashing between the ln-only and exp-only sets. Set
    ids/ordering are preserved — only which sets the chooser may pick for
    ln/exp changes; the chosen set genuinely contains both functions."""
    import concourse.bacc as _bacc_mod

    if getattr(_bacc_mod, "_act_tables_patched", False):
        return
    orig = _bacc_mod.get_activation_tables
    Ln = mybir.ActivationFunctionType.Ln
    Exp = mybir.ActivationFunctionType.Exp

    def patched(arch):
        tables = dict(orig(arch))
        if any(
            Ln in fns and Exp in fns for name, fns in tables.items()
        ):
            tables = {
                name: (
                    fns
                    if (Ln in fns and Exp in fns)
                    else fns - {Ln, Exp}
                )
                for name, fns in tables.items()
            }
        return tables

    _bacc_mod.get_activation_tables = patched
    _bacc_mod._act_tables_patched = True


_patch_act_tables()

# ---------------------------------------------------------------------------
# Fast path: geometric chain (uniform gamma, uniform mu spacing)
# ---------------------------------------------------------------------------
BH = N // 2             # 384 b's per core (b-half)
AQ = N // 4             # 192 a's per core (a-quarter)
NBT = BH // 128         # 3 b-tiles
KAUG = 15               # 5 aug rows x (hi,hi,lo)/(hi,lo,hi) bf16 split
EPS_LN = 3e-4           # ln bias; > PE accumulation residue
RBASE = 4               # second chain base index


def _build_fast(mu0_zero: bool, kP: float, kBase: float, k0: float):
    """Runtime scalars (kP = 2*delta*sqrt(gamma) etc.) are baked as
    immediates; the build cache is keyed by their values."""
    nc = bacc.Bacc("TRN2", target_bir_lowering=False, debug=False)
    gB = nc.dram_tensor("gB", [KAUG, BH], BF16, kind="ExternalInput")
    gA = nc.dram_tensor("gA", [KAUG, AQ], BF16, kind="ExternalInput")
    # fpw[b, t*128 + r*16 + i] = (feat @ (W_r * c_r / sqrt(n)).T)[b_global, i]
    fpw = nc.dram_tensor("fpw", [128, NBT * R * COUT], BF16, kind="ExternalInput")
    outt = nc.dram_tensor("outt", [COUT, AQ], F32, kind="ExternalOutput")

    Exp = mybir.ActivationFunctionType.Exp
    Ln = mybir.ActivationFunctionType.Ln

    with tile.TileContext(nc) as tc:
        with (
            tc.tile_pool(name="const", bufs=1) as const,
            tc.tile_pool(name="psd", bufs=1, space="PSUM") as psd,
            tc.tile_pool(name="pso", bufs=1, space="PSUM") as pso,
        ):
            gb_sb = const.tile([KAUG, BH], BF16)
            ga_sb = const.tile([KAUG, AQ], BF16)
            fp_sb = const.tile([128, NBT * R * COUT], BF16)
            eps_sb = const.tile([128, 1], F32)
            nc.vector.memset(eps_sb[:], EPS_LN)
            tall = const.tile([128, R, NBT, AQ], BF16)
            ln_sb = const.tile([128, NBT, AQ], F32)
            d_sb = const.tile([128, NBT, AQ], F32)
            u4_sb = const.tile([128, NBT, AQ], F32)
            p_sb = const.tile([128, NBT, AQ], BF16)

            nc.gpsimd.dma_start(out=ga_sb[:], in_=gA.ap())
            nc.sync.dma_start(out=gb_sb[:], in_=gB.ap())
            nc.gpsimd.dma_start(out=fp_sb[:], in_=fpw.ap())

            # s = gamma * d^2 (b-side aug rows pre-scaled by gamma on host)
            # one padded PSUM bank per b-tile so each matmul stays in-bank
            s_ps = psd.tile([128, NBT, 512], F32)
            for t in range(NBT):
                nc.tensor.matmul(
                    out=s_ps[:, t, :AQ],
                    lhsT=gb_sb[:, ts(t, 128)],
                    rhs=ga_sb[:],
                    start=True,
                    stop=True,
                )
            s_view = s_ps[:, :, :AQ]

            # ACT passes, all from the natural_log_exp table set:
            # L = ln(s + eps); d = exp(0.5 L) = sqrt(s); P = exp(kP * d)
            nc.scalar.activation(out=ln_sb[:], in_=s_view, func=Ln, bias=eps_sb[:])
            if mu0_zero:
                # T_0 = exp(-s)
                nc.scalar.activation(out=tall[:, 0], in_=s_view, func=Exp, scale=-1.0)
            nc.scalar.activation(out=d_sb[:], in_=ln_sb[:], func=Exp, scale=0.5)
            nc.scalar.activation(out=p_sb[:], in_=d_sb[:], func=Exp, scale=kP)
            # second base: T_4 = exp(kBase * d - s)
            nc.vector.scalar_tensor_tensor(
                out=u4_sb[:],
                in0=d_sb[:],
                scalar=kBase,
                in1=s_view,
                op0=mybir.AluOpType.mult,
                op1=mybir.AluOpType.subtract,
            )
            nc.scalar.activation(out=tall[:, RBASE], in_=u4_sb[:], func=Exp)
            if not mu0_zero:
                # T_0 = exp(k0 * d - s)
                u0_sb = const.tile([128, NBT, AQ], F32)
                nc.vector.scalar_tensor_tensor(
                    out=u0_sb[:],
                    in0=d_sb[:],
                    scalar=k0,
                    in1=s_view,
                    op0=mybir.AluOpType.mult,
                    op1=mybir.AluOpType.subtract,
                )
                nc.scalar.activation(out=tall[:, 0], in_=u0_sb[:], func=Exp)

            # geometric chains from the two bases
            for dst, src in ((1, 0), (2, 1), (3, 2), (5, 4), (6, 5), (7, 6)):
                nc.vector.tensor_mul(tall[:, dst], tall[:, src], p_sb[:])

            # out^T[i, a] = sum_{r, b} fp[b, (r,i)] * T_r[b, a]
            out_ps = pso.tile([COUT, AQ], F32)
            order = (0, 1, 2, RBASE, 3, 5, 6, 7) if RBASE == 4 else tuple(range(R))
            k = 0
            for r in order:
                for t in range(NBT):
                    nc.tensor.matmul(
                        out=out_ps[:],
                        lhsT=fp_sb[:, t * R * COUT + r * COUT : t * R * COUT + (r + 1) * COUT],
                        rhs=tall[:, r, t],
                        start=(k == 0),
                        stop=(k == R * NBT - 1),
                    )
                    k += 1

            res_sb = const.tile([COUT, AQ], F32)
            nc.scalar.copy(out=res_sb[:], in_=out_ps[:])
            nc.sync.dma_start(out=outt.ap(), in_=res_sb[:])

    nc.compile()
    return nc


def _split8(x):
    """Split x = hi + lo into two bf16 parts (products of parts are exact
    in f32)."""
    x = x.astype(np.float32)
    hi = x.astype(NPBF16).astype(np.float32)
    lo = (x - hi).astype(NPBF16).astype(np.float32)
    return hi, lo


def _kernel_fast(f, g, Wf, muf, gaf, nn):
    global LAST_EXEC_NS, LAST_RESULTS
    ga = float(gaf[0])
    delta = float(muf[1] - muf[0])
    mu0_zero = abs(float(muf[0])) < 1e-7

    kP = float(np.float32(2.0 * delta * np.sqrt(ga)))
    kBase = float(np.float32(2.0 * np.sqrt(ga) * float(muf[RBASE])))
    k0 = float(np.float32(2.0 * np.sqrt(ga) * float(muf[0])))

    key = ("fast", mu0_zero, kP, kBase, k0)
    if key not in _CACHE:
        _CACHE[key] = _build_fast(mu0_zero, kP, kBase, k0)
    nc = _CACHE[key]

    gs = g.astype(np.float32)
    sq = (gs * gs).sum(1)
    one = np.ones(N, np.float32)
    augb5 = np.stack([gs[:, 0], gs[:, 1], gs[:, 2], sq, one]) * np.float32(ga)
    auga5 = np.stack([-2 * gs[:, 0], -2 * gs[:, 1], -2 * gs[:, 2], one, sq])
    bh, bl = _split8(augb5)
    ah, al = _split8(auga5)
    augb = np.concatenate([bh, bh, bl], axis=0)          # [15, N]
    auga = np.concatenate([ah, al, ah], axis=0)

    # weights with per-r chain constants folded in
    # rbf_r = T_r * exp(-ga*(mu0 + r*delta)^2 + 2*ga*mu0*... ) — with the
    # chain T_r = T_0 * P^r and T_0 = exp(2*ga*mu0*d - s), the fold is
    # c_r = exp(-ga*mu_r^2 + ga*mu0^2 + ... ); derive directly:
    # T_r = exp(-ga*d^2 + 2*ga*(mu0 + r*delta)*d) = rbf_r * exp(ga*mu_r^2)
    # => c_r = exp(-ga*mu_r^2) / sqrt(n_norm)
    Wc = np.empty((R, COUT, CIN), np.float64)
    for r in range(R):
        c = np.exp(-ga * float(muf[r]) ** 2) / np.sqrt(nn)
        Wc[r] = Wf.astype(np.float64)[r] * c

    # fp[b_global, r, i] = feat @ Wc[r].T
    fp_full = np.einsum("bj,rij->bri", f.astype(np.float64), Wc).astype(np.float32)

    in_maps = []
    for h in range(2):
        fp_h = fp_full[h * BH : (h + 1) * BH].reshape(NBT, 128, R * COUT)
        fp_blob = np.ascontiguousarray(
            fp_h.transpose(1, 0, 2).reshape(128, NBT * R * COUT)
        ).astype(NPBF16)
        gB = np.ascontiguousarray(augb[:, h * BH : (h + 1) * BH]).astype(NPBF16)
        for q in range(4):
            gA = np.ascontiguousarray(auga[:, q * AQ : (q + 1) * AQ]).astype(NPBF16)
            in_maps.append({"gB": gB, "gA": gA, "fpw": fp_blob})

    trace = os.environ.get("KERNEL_TRACE", "0") == "1"
    res = run_bass_kernel_spmd(nc, in_maps, core_ids=list(range(NCORES)), trace=trace)
    LAST_EXEC_NS = res.exec_time_ns
    LAST_RESULTS = res

    out = np.zeros((1, N, COUT), np.float32)
    for q in range(4):
        acc = res.results[q]["outt"].astype(np.float64) + res.results[4 + q][
            "outt"
        ].astype(np.float64)
        out[0, q * AQ : (q + 1) * AQ, :] = acc.T.astype(np.float32)
    return out


# ---------------------------------------------------------------------------
# Fallback path: original per-r kernel (arbitrary mu/gamma)
# ---------------------------------------------------------------------------
AHALF = N // 2          # 384 output points per a-half
RPC = 2                 # radial bases per core
NBT6 = N // 128         # 6 b-tiles
KAUG15 = 15
EPS_BIAS = 3e-5


def _build_legacy():
    nc = bacc.Bacc("TRN2", target_bir_lowering=False, debug=False)
    geo = nc.dram_tensor("geo", [KAUG15, N + AHALF], F32, kind="ExternalInput")
    feats = nc.dram_tensor("feats", [CIN, N + RPC * COUT], F32, kind="ExternalInput")
    scols = nc.dram_tensor("scols", [128, 2 * RPC], F32, kind="ExternalInput")
    outt = nc.dram_tensor("outt", [COUT, AHALF], F32, kind="ExternalOutput")

    with tile.TileContext(nc) as tc:
        with (
            tc.tile_pool(name="const", bufs=1) as const,
            tc.tile_pool(name="work", bufs=3) as work,
            tc.tile_pool(name="psd", bufs=2, space="PSUM") as psd,
            tc.tile_pool(name="psf", bufs=1, space="PSUM") as psf,
            tc.tile_pool(name="pso", bufs=1, space="PSUM") as pso,
        ):
            geo_sb = const.tile([KAUG15, N + AHALF], F32R)
            feats_sb = const.tile([CIN, N + RPC * COUT], F32)
            scols_sb = const.tile([128, 2 * RPC], F32)
            eps_sb = const.tile([128, 1], F32)
            nc.vector.memset(eps_sb[:], EPS_BIAS)
            nc.sync.dma_start(out=geo_sb[:], in_=geo.ap().bitcast(F32R))
            nc.scalar.dma_start(out=feats_sb[:], in_=feats.ap())
            nc.scalar.dma_start(out=scols_sb[:], in_=scols.ap())
            augb_sb = geo_sb[:, :N]
            auga_sb = geo_sb[:, N:]
            featt_sb = feats_sb[:, :N]
            wtg_sb = feats_sb[:, N:]

            d_sb = const.tile([128, NBT6, AHALF], F32)
            for tp in range(NBT6 // 2):
                d2_ps = psd.tile([128, 2, 512], F32)
                for j in range(2):
                    nc.tensor.matmul(
                        out=d2_ps[:, j, :AHALF],
                        lhsT=augb_sb[:, ts(2 * tp + j, 128)],
                        rhs=auga_sb[:],
                        start=True,
                        stop=True,
                    )
                nc.scalar.activation(
                    out=d_sb[:, 2 * tp : 2 * tp + 2, :],
                    in_=d2_ps[:, :, :AHALF],
                    func=mybir.ActivationFunctionType.Sqrt,
                    bias=eps_sb[:],
                    scale=1.0,
                )

            fp_ps = psf.tile([128, NBT6, RPC * COUT], F32)
            for t in range(NBT6):
                nc.tensor.matmul(
                    out=fp_ps[:, t, :],
                    lhsT=featt_sb[:, ts(t, 128)],
                    rhs=wtg_sb[:],
                    start=True,
                    stop=True,
                )
            fp_sb = const.tile([128, NBT6, RPC * COUT], BF16)
            nc.vector.tensor_copy(out=fp_sb[:], in_=fp_ps[:])

            out_ps = pso.tile([COUT, AHALF], F32)
            k = 0
            for tp in range(NBT6 // 2):
                t_bf = work.tile([128, 2, RPC, AHALF], BF16, tag="t_bf")
                for rl in range(RPC):
                    nc.vector.tensor_scalar(
                        out=t_bf[:, :, rl, :],
                        in0=d_sb[:, 2 * tp : 2 * tp + 2, :],
                        scalar1=scols_sb[:, 2 * rl : 2 * rl + 1],
                        scalar2=scols_sb[:, 2 * rl + 1 : 2 * rl + 2],
                        op0=mybir.AluOpType.subtract,
                        op1=mybir.AluOpType.mult,
                    )
                q_bf = work.tile([128, 2, RPC, AHALF], BF16, tag="q_bf")
                nc.vector.tensor_mul(q_bf[:], t_bf[:], t_bf[:])
                rbf = work.tile([128, 2, RPC, AHALF], BF16, tag="rbf")
                nc.scalar.activation(
                    out=rbf[:],
                    in_=q_bf[:],
                    func=mybir.ActivationFunctionType.Exp,
                    scale=-1.0,
                )
                for j in range(2):
                    for rl in range(RPC):
                        nc.tensor.matmul(
                            out=out_ps[:],
                            lhsT=fp_sb[:, 2 * tp + j, ts(rl, COUT)],
                            rhs=rbf[:, j, rl, :],
                            start=(k == 0),
                            stop=(k == NBT6 * RPC - 1),
                        )
                        k += 1

            res_sb = const.tile([COUT, AHALF], F32)
            nc.vector.tensor_copy(out=res_sb[:], in_=out_ps[:])
            nc.sync.dma_start(out=outt.ap(), in_=res_sb[:])

    nc.compile()
    return nc


def _split12(x):
    """Veltkamp split: x = hi + lo with hi having <=12 significant bits."""
    x = x.astype(np.float32)
    c = (np.float32(2.0**12 + 1.0) * x).astype(np.float32)
    hi = (c - (c - x).astype(np.float32)).astype(np.float32)
    lo = (x - hi).astype(np.float32)
    return hi, lo


def _kernel_legacy(f, g, Wf, muf, gaf, nn):
    global LAST_EXEC_NS, LAST_RESULTS
    sq = (g * g).sum(1, dtype=np.float32)
    one = np.ones(N, np.float32)
    augb5 = np.stack([g[:, 0], g[:, 1], g[:, 2], sq, one])
    auga5_full = np.stack([-2 * g[:, 0], -2 * g[:, 1], -2 * g[:, 2], one, sq])
    bh, bl = _split12(augb5)
    ah, al = _split12(auga5_full)
    augb = np.concatenate([bh, bh, bl], axis=0)
    auga_full = np.concatenate([ah, al, ah], axis=0)
    featt = np.ascontiguousarray(f.T)
    Wn = (Wf.astype(np.float64) / np.sqrt(nn)).astype(np.float32)
    sg = np.sqrt(gaf)

    if "legacy" not in _CACHE:
        _CACHE["legacy"] = _build_legacy()
    nc = _CACHE["legacy"]

    in_maps = []
    for c in range(NCORES):
        h, grp = c // 4, c % 4
        rr = [RPC * grp + j for j in range(RPC)]
        wtg = np.ascontiguousarray(
            np.concatenate([Wn[r].T for r in rr], axis=1).astype(np.float32)
        )
        scol_vals = []
        for r in rr:
            scol_vals += [muf[r], sg[r]]
        scols = np.ascontiguousarray(
            np.tile(np.asarray(scol_vals, np.float32), (128, 1))
        )
        geo = np.ascontiguousarray(
            np.concatenate([augb, auga_full[:, h * AHALF : (h + 1) * AHALF]], axis=1)
        )
        feats_blob = np.ascontiguousarray(np.concatenate([featt, wtg], axis=1))
        in_maps.append({"geo": geo, "feats": feats_blob, "scols": scols})

    trace = os.environ.get("KERNEL_TRACE", "0") == "1"
    res = run_bass_kernel_spmd(nc, in_maps, core_ids=list(range(NCORES)), trace=trace)
    LAST_EXEC_NS = res.exec_time_ns
    LAST_RESULTS = res

    out = np.zeros((1, N, COUT), np.float32)
    for h in range(2):
        acc = np.zeros((COUT, AHALF), np.float64)
        for grp in range(4):
            acc += res.results[h * 4 + grp]["outt"].astype(np.float64)
        out[0, h * AHALF : (h + 1) * AHALF, :] = acc.T.astype(np.float32)
    return out


def kernel(features, geometry, W, mu, gamma, n_norm):
    f = np.ascontiguousarray(np.asarray(features, np.float32)[0])      # [N, CIN]
    g = np.ascontiguousarray(np.asarray(geometry, np.float32)[0])      # [N, 3]
    Wf = np.asarray(W, np.float32)                                     # [R, COUT, CIN]
    muf = np.asarray(mu, np.float64)
    gaf = np.asarray(gamma, np.float64)
    nn = float(np.asarray(n_norm))

    diffs = np.diff(muf)
    uniform = (
        gaf.min() > 0
        and np.ptp(gaf) <= 1e-5 * abs(gaf[0])
        and diffs.min() > 0
        and np.ptp(diffs) <= 1e-4 * abs(diffs[0])
    )
    if uniform:
        return _kernel_fast(f, g, Wf, muf, gaf, nn)
    return _kernel_legacy(f, g, Wf, muf, gaf, nn)
